# revision 4
# baseline (speedup 1.0000x reference)
"""Trainium2 Bass kernel for nn_CONCATNet_7447473291796 (gnn_message_passing).

Strategy (pure data parallelism, 16 batches per core across 8 cores):
  Only 66 of the 4096 wafer rows per batch are ever used, so the kernel
  gathers them straight from the HBM-resident batch shard with SWDGE
  dma_gather in bf16 16-bit transpose mode: each gathered tile lands
  already transposed ([d x rows]) and feeds the PE directly.

  Everything else is restructured around weight-stationary matmuls:
    - pm^T[dout, prow] accumulates stage/wafer/dyn contributions with the
      (tiny) weights as the stationary lhsT and 256-512 wide moving rhs.
      The host un-transposes the bf16 result.
    - the robot-arm a_loc path is algebraically folded through
      W_concat @ W_robot[0:D] on the host, so arm embeddings need no
      selection matmuls, no PE transpose and no dependency on pm at all.
    - stage rows (encoded_col is only [16,32,128] per core) and all
      scalar preprocessing (remain_prs, dyn vectors) are prepared host-side
      and shipped as small dense inputs.

  idx layout: 4 gather calls x 384 idx (256 wafer + 8 a_loc + 8 recipe +
  112 pad), one per table quarter of 4 batches (sixteen zero rows are
  interleaved after each quarter: int16 idx caps a table slice at 32768
  rows, and the zero row doubles as the pad / invalid-loc target).

All per-core variation is data staged through DRAM inputs; the Bass
program is identical on every core.
"""

import numpy as np
import ml_dtypes

import concourse.bass as bass
import concourse.bacc as bacc
import concourse.mybir as mybir
import concourse.tile as tile
from concourse import library_config
from concourse.bass_utils import run_bass_kernel_spmd

B, N, S, P, D = 128, 4096, 32, 64, 128
NORM = 300.0
NCORES = 8
BL = B // NCORES          # local batches per core = 16
NCALL = 4                 # gather calls (= SWDGE queues)
QB = BL // NCALL          # batches per gather call = 4
GIDX = 384                # idxs per call: 256 wafer + 8 a_loc + 8 recipe + 112 pad
QROWS = QB * N + 16       # table rows per quarter (incl 16 zero rows)
ZIDX = QB * N             # zero-row index within a quarter
NARM = 2 * BL             # arm rows per core = 32
XCOLS = 8 * 128 + 2 * NARM  # xstageT cols: 1024 pm + 32 a_loc-stage + 32 ns

F32 = mybir.dt.float32
BF16 = mybir.dt.bfloat16
I16 = mybir.dt.int16
BF = ml_dtypes.bfloat16

_prog_cache = None


def _wrap16(idx_flat: np.ndarray) -> np.ndarray:
    """Logical index list -> [128, n//16] int16 SWDGE layout (idx i lives at
    [i % 16, i // 16], replicated into all 8 16-partition groups)."""
    n = idx_flat.shape[0]
    assert n % 16 == 0
    a = idx_flat.astype(np.int16).reshape(n // 16, 16).T
    return np.tile(a, (8, 1))


def _build_program():
    nc = bacc.Bacc("TRN2", target_bir_lowering=False, num_swdge_queues=4,
                   debug=False)

    rows_h = nc.declare_dram_parameter("rows", [NCALL * QROWS, D], BF16,
                                       isOutput=False)
    xstage_h = nc.declare_dram_parameter("xstage", [128, XCOLS], BF16,
                                         isOutput=False)
    wstack_h = nc.declare_dram_parameter("wstack", [128, 6, D], BF16,
                                         isOutput=False)
    vecs_h = nc.declare_dram_parameter("vecs", [1, 1472], BF16, isOutput=False)
    idx_h = nc.declare_dram_parameter("idx", [128, NCALL * (GIDX // 16)], I16,
                                      isOutput=False)

    out_pm_h = nc.declare_dram_parameter("out_pm", [128, 8 * 128], BF16,
                                         isOutput=True)
    out_arm_h = nc.declare_dram_parameter("out_arm", [NARM, D], BF16,
                                          isOutput=True)

    with tile.TileContext(nc) as tc:
        with (
            tc.tile_pool(name="consts", bufs=1) as cpool,
            tc.tile_pool(name="gathers", bufs=1) as gpool,
            tc.tile_pool(name="outs", bufs=1) as opool,
            tc.tile_pool(name="ps_pm", bufs=2, space="PSUM") as ps_pm,
            tc.tile_pool(name="ps_arm", bufs=1, space="PSUM") as ps_arm,
        ):
            # ---- gathers in flight first: ucode lib, idx, 4 gather calls ----
            nc.gpsimd.load_library(library_config.mlp)
            idx = cpool.tile([128, NCALL * (GIDX // 16)], I16, name="idx")
            nc.sync.dma_start(out=idx[:], in_=idx_h[:])

            nI = GIDX // 16
            gt = []
            for q in range(NCALL):
                g = gpool.tile([128, 1, GIDX], BF16, name=f"g{q}", uniquify=False)
                nc.gpsimd.dma_gather(
                    g[:], rows_h[q * QROWS : (q + 1) * QROWS, :],
                    idx[:, q * nI : (q + 1) * nI],
                    GIDX, GIDX, D, transpose=True, queue_num=q,
                )
                gt.append(g)

            # ---- dense loads: xstageT behind idx on SP; weights on ACT ----
            xst = cpool.tile([128, XCOLS], BF16, name="xst")
            nc.sync.dma_start(out=xst[:], in_=xstage_h[:])
            wsb = cpool.tile([128, 6, D], BF16, name="wsb")
            nc.scalar.dma_start(out=wsb[:], in_=wstack_h[:])
            vecs = cpool.tile([1, 1472], BF16, name="vecs")
            nc.scalar.dma_start(out=vecs[:], in_=vecs_h[:])

            w_cs = wsb[:, 0, :]       # W_concat stage segment   [d, dout]
            w_cw = wsb[:, 1, :]       # W_concat wafer segment
            w_rw = wsb[:, 2, :]       # W_robot wafer segment
            w_rn = wsb[:, 3, :]       # W_robot next-stage segment
            w_fs = wsb[:, 4, :]       # W_concat[0:D]   @ W_robot[0:D]
            w_fw = wsb[:, 5, :]       # W_concat[D:2D]  @ W_robot[0:D]
            rflat = vecs[:, 0:1024]           # remain_prs, pmT col order
            rfa = vecs[:, 1024:1056]          # remain_prs at each arm's loc PM
            flag = vecs[:, 1056:1088]         # 1.0 where arm loc == P+1
            v_dyn = vecs[:, 1088:1216]        # W_dyn[0] @ W_concat[2D:3D]
            v_dyn_rl = vecs[:, 1216:1344]     # v_dyn @ W_robot[0:D]
            wrl_sum = vecs[:, 1344:1472]      # column sums of W_robot[0:D]

            # ---- pm^T halves: [dout=128, 512 prow] psum each ----
            pm_sb = opool.tile([128, 8 * 128], BF16, name="pm_sb")
            for h in range(2):
                pmp = ps_pm.tile([128, 512], F32, name=f"pmp{h}", tag="pmp")
                cols = slice(h * 512, (h + 1) * 512)
                nc.tensor.matmul(pmp[:], lhsT=w_cs, rhs=xst[:, cols],
                                 start=True, stop=False)
                nc.tensor.matmul(pmp[:], lhsT=v_dyn, rhs=rflat[:, cols],
                                 start=False, stop=False)
                for qq in range(2):
                    q = 2 * h + qq
                    nc.tensor.matmul(
                        pmp[:, qq * 256 : (qq + 1) * 256], lhsT=w_cw,
                        rhs=gt[q][:, 0, 0:256],
                        start=False, stop=(qq == 1), skip_group_check=True,
                    )
                if h == 0:
                    nc.vector.tensor_copy(out=pm_sb[:, cols], in_=pmp[:])
                else:
                    nc.scalar.copy(out=pm_sb[:, cols], in_=pmp[:])

            # ---- arm rows: armp[arm, dout], no dependency on pm ----
            # stage the 4 calls' arm cols into one contiguous lhsT tile
            # (PE psum writes need base partition 0/32/64, so no per-call
            # 8-row matmuls)
            aw = opool.tile([128, 2, NARM], BF16, name="aw")
            for q in range(NCALL):
                src = gt[q][:, 0, 256:272].rearrange("p (t a) -> p t a", t=2)
                if q % 2 == 0:
                    nc.vector.tensor_copy(out=aw[:, :, 8 * q : 8 * q + 8], in_=src)
                else:
                    nc.scalar.copy(out=aw[:, :, 8 * q : 8 * q + 8], in_=src)
            armp = ps_arm.tile([NARM, D], F32, name="armp", tag="armp")
            nc.tensor.matmul(armp[:], lhsT=xst[:, 1024:1056], rhs=w_fs,
                             start=True, stop=False)
            nc.tensor.matmul(armp[:], lhsT=xst[:, 1056:1088], rhs=w_rn,
                             start=False, stop=False)
            nc.tensor.matmul(armp[:], lhsT=rfa, rhs=v_dyn_rl,
                             start=False, stop=False)
            nc.tensor.matmul(armp[:], lhsT=aw[:, 0, :], rhs=w_fw,
                             start=False, stop=False)
            nc.tensor.matmul(armp[:], lhsT=aw[:, 1, :], rhs=w_rw,
                             start=False, stop=False)
            nc.tensor.matmul(armp[:], lhsT=flag, rhs=wrl_sum,
                             start=False, stop=True)
            arm_sb = opool.tile([NARM, D], BF16, name="arm_sb")
            nc.vector.tensor_copy(out=arm_sb[:], in_=armp[:])

            nc.sync.dma_start(out=out_pm_h[:], in_=pm_sb[:])
            nc.scalar.dma_start(out=out_arm_h[:], in_=arm_sb[:])

    nc.compile()
    return nc


def _get_program():
    global _prog_cache
    if _prog_cache is None:
        _prog_cache = _build_program()
    return _prog_cache


def _prep_core(c, rows_bf, col_bf, remain, W, loc_hold_wafer, loc_stage,
               robot_arm1_loc, robot_arm2_loc, arm1_recipe, arm2_recipe,
               arm1_next_stage, arm2_next_stage):
    b0 = c * BL
    bs = slice(b0, b0 + BL)

    # quartered wafer table with 16 zero rows after each quarter
    rows = np.zeros((NCALL * QROWS, D), BF)
    for q in range(NCALL):
        rows[q * QROWS : q * QROWS + QB * N] = (
            rows_bf[b0 + q * QB : b0 + (q + 1) * QB].reshape(QB * N, D)
        )

    lhw = np.where(loc_hold_wafer[bs] >= 0, loc_hold_wafer[bs], 0).astype(np.int64)
    lst = loc_stage[bs].astype(np.int64)                      # in [1, S]
    rem = remain[bs]                                          # [BL, P] f32
    loc = np.stack([robot_arm1_loc[bs, 0], robot_arm2_loc[bs, 0]], 1).astype(np.int64)
    rec = np.stack([arm1_recipe[bs, 0], arm2_recipe[bs, 0]], 1).astype(np.int64)
    rec = np.where(rec >= 0, rec, 0)
    nst = np.stack([arm1_next_stage[bs, 0], arm2_next_stage[bs, 0]], 1).astype(np.int64)

    locv = (loc >= 1) & (loc <= P)                            # [BL, 2] valid pm loc
    locp = np.where(locv, loc - 1, 0)                         # the arm's PM index
    lbi = np.arange(BL)[:, None]

    # gather idx: per call q (batches 4q..4q+3): 256 wafer, 8 a_loc, 8 recipe, pad
    qb = np.arange(QB)[:, None]
    idx_parts = []
    for q in range(NCALL):
        bq = slice(q * QB, (q + 1) * QB)
        wafer = (qb * N + lhw[bq]).reshape(-1)                # [256] block order
        aloc_w = np.where(locv[bq], qb * N + lhw[lbi[bq], locp[bq]], ZIDX).reshape(-1)
        recipe = (qb * N + rec[bq]).reshape(-1)
        pad = np.full(GIDX - QB * P - 4 * QB, ZIDX, np.int64)
        idx_parts.append(_wrap16(np.concatenate([wafer, aloc_w, recipe, pad])))
    idx = np.concatenate(idx_parts, axis=1)

    # xstageT [128, 1088]: cols = per-pm stage rows (pmT order), a_loc stage, ns
    colc = col_bf[bs]                                         # [BL, S, D] bf16
    xst = np.zeros((XCOLS, D), BF)
    xst[0:1024] = colc[lbi, lst - 1].reshape(1024, D)
    a_st = np.where(locv[:, :, None], colc[lbi, lst[lbi, locp] - 1], 0).reshape(NARM, D)
    xst[1024:1056] = a_st
    nsv = (nst >= 1) & (nst <= S)
    xst[1056:1088] = np.where(nsv[:, :, None],
                              colc[lbi, np.where(nsv, nst - 1, 0)], 0
                              ).reshape(NARM, D)

    vecs = np.zeros((1, 1472), BF)
    vecs[0, 0:1024] = rem.reshape(-1).astype(BF)
    vecs[0, 1024:1056] = np.where(locv, rem[lbi, locp], 0).reshape(-1).astype(BF)
    vecs[0, 1056:1088] = (loc == P + 1).reshape(-1).astype(BF)
    vecs[0, 1088:1472] = W["vec3"]

    return {
        "rows": rows,
        "xstage": np.ascontiguousarray(xst.T),
        "wstack": W["wstack"],
        "vecs": vecs,
        "idx": idx,
    }


def make_in_maps(inputs):
    inputs = {k: np.asarray(v) for k, v in inputs.items()}
    Wc = inputs["W_concat"].astype(np.float32)
    Wr = inputs["W_robot"].astype(np.float32)
    Wd = inputs["W_dyn"].astype(np.float32)
    w_rl = Wr[0:D]

    wstack = np.ascontiguousarray(
        np.stack(
            [Wc[0:D], Wc[D : 2 * D], Wr[D : 2 * D], Wr[2 * D : 3 * D],
             Wc[0:D] @ w_rl, Wc[D : 2 * D] @ w_rl],
            axis=1,
        )
    ).astype(BF)                                              # [128, 6, D]
    v_dyn = (Wd[0:1] @ Wc[2 * D : 3 * D]).reshape(D)
    vec3 = np.concatenate([v_dyn, v_dyn @ w_rl, w_rl.sum(0)]).astype(BF)
    W = {"wstack": wstack, "vec3": vec3}

    rows_bf = inputs["encoded_row"].astype(BF)                # [B, N, D]
    col_bf = inputs["encoded_col"].astype(BF)                 # [B, S, D]
    clk = inputs["clock"].astype(np.float32)                  # [B, 1]
    lpet = inputs["loc_process_end_time"].astype(np.float32)  # [B, P]
    remain = np.maximum(lpet - clk, 0.0) / NORM               # [B, P]

    ks = ("loc_hold_wafer", "loc_stage", "robot_arm1_loc", "robot_arm2_loc",
          "arm1_recipe", "arm2_recipe", "arm1_next_stage", "arm2_next_stage")
    return [
        _prep_core(c, rows_bf, col_bf, remain, W, **{k: inputs[k] for k in ks})
        for c in range(NCORES)
    ]


def assemble_output(res):
    out = np.empty((B, P + 2, D), np.float32)
    for c in range(NCORES):
        pmT = res[c]["out_pm"].astype(np.float32)             # [128, 1024]
        pm = pmT.reshape(D, 8, 2, P).transpose(1, 2, 3, 0).reshape(BL, P, D)
        out[c * BL : (c + 1) * BL, 0:P, :] = pm
        out[c * BL : (c + 1) * BL, P:, :] = (
            res[c]["out_arm"].astype(np.float32).reshape(BL, 2, D)
        )
    return out


def kernel(**inputs):
    in_maps = make_in_maps(inputs)
    nc = _get_program()
    res = run_bass_kernel_spmd(nc, in_maps, list(range(NCORES))).results
    return assemble_output(res)
